# revision 1
# baseline (speedup 1.0000x reference)
"""ContextAttention Trainium2 kernel (8-core data parallel).

Computation (per batch row b, S=20, D=300, J=512):
  valid = cumprod(labels != 0)                      prefix-valid mask
  fea   = guide[ann[2b]]                            (host gather, pure data movement)
  pre[s,:] = ctx[b,s,:] @ W_sent.T + b_sent + b_emb + valid[b,s]*(fea @ W_emb.T)
  H = tanh(pre);  scores = H @ w_fc                 (b_fc dropped: softmax shift-invariant)
  attn = renorm(softmax(scores) * (labels != 0))
  out[b,:] = sum_s attn[s] * embedded[b,s,:]

Device layout: s-major, 128-batch tiles. Contraction-dim-major ("transposed")
context/fea prepared on host so the PE streams natural tiles; the valid-gated
guidance add and the attn-weighted sum both run as diagonal-matrix matmuls
accumulated in PSUM.
"""

import sys
from contextlib import ExitStack

import numpy as np

if "/opt/trn_rl_repo" not in sys.path:
    sys.path.append("/opt/trn_rl_repo")

import concourse.bass as bass
import concourse.tile as tile
from concourse import bacc, mybir
from concourse.bass_utils import run_bass_kernel_spmd

B, S, D, J, VG = 8192, 20, 300, 512, 2078
NCORES = 8
BC = B // NCORES          # 1024 batch rows per core
NBT = BC // 128           # 8 batch tiles per core
DX = D + 1                # ones row appended for fused bias
VGP = 2176                # VG padded to 17*128
NK = VGP // 128           # 17 contraction chunks for fea @ W_emb.T
DP = [(0, 128), (128, 128), (256, DX - 256)]  # contraction chunks for ctx @ W_sent.T
SH = 10                   # s-halves for SBUF footprint
F32 = mybir.dt.float32

_NC_CACHE = {}


def _build(mm_dt):
    nc = bacc.Bacc("TRN2", target_bir_lowering=False, debug=False)
    MMD = mm_dt

    ctx_d = nc.dram_tensor("ctx", [NBT, DX, S, 128], MMD, kind="ExternalInput").ap()
    emb_d = nc.dram_tensor("emb", [BC, S * D], MMD, kind="ExternalInput").ap()
    fea_d = nc.dram_tensor("feaT", [NBT, 128, NK, 128], MMD, kind="ExternalInput").ap()
    lab_d = nc.dram_tensor("lab", [NBT, 128, S], F32, kind="ExternalInput").ap()
    wst_d = nc.dram_tensor("wst", [DX, J], MMD, kind="ExternalInput").ap()
    wet_d = nc.dram_tensor("wet", [NK, 128, J], MMD, kind="ExternalInput").ap()
    wfc_d = nc.dram_tensor("wfc", [128, J], F32, kind="ExternalInput").ap()
    eye_d = nc.dram_tensor("eye", [128, S * 128], MMD, kind="ExternalInput").ap()
    out_d = nc.dram_tensor("wc", [NBT, 128, D], F32, kind="ExternalOutput").ap()

    mul = mybir.AluOpType.mult
    add = mybir.AluOpType.add

    with tile.TileContext(nc) as tc, ExitStack() as ctx:
        consts = ctx.enter_context(tc.tile_pool(name="consts", bufs=1))
        ctxp = ctx.enter_context(tc.tile_pool(name="ctxp", bufs=3))
        feap = ctx.enter_context(tc.tile_pool(name="feap", bufs=2))
        fep = ctx.enter_context(tc.tile_pool(name="fep", bufs=2))
        embp = ctx.enter_context(tc.tile_pool(name="embp", bufs=1))
        hp = ctx.enter_context(tc.tile_pool(name="hp", bufs=4))
        dgp = ctx.enter_context(tc.tile_pool(name="dgp", bufs=1))
        sm = ctx.enter_context(tc.tile_pool(name="sm", bufs=2))
        outp = ctx.enter_context(tc.tile_pool(name="outp", bufs=2))
        ps_fe_p = ctx.enter_context(tc.tile_pool(name="psfe", bufs=2, space="PSUM"))
        ps_h_p = ctx.enter_context(tc.tile_pool(name="psh", bufs=3, space="PSUM"))
        ps_wc_p = ctx.enter_context(tc.tile_pool(name="pswc", bufs=2, space="PSUM"))

        wet_sb = []
        for k in range(NK):
            t = consts.tile([128, J], MMD, tag=f"wet{k}")
            nc.sync.dma_start(out=t, in_=wet_d[k])
            wet_sb.append(t)
        wst_sb = []
        for i, (o, p) in enumerate(DP):
            t = consts.tile([p, J], MMD, tag=f"wst{i}")
            nc.sync.dma_start(out=t, in_=wst_d[o : o + p])
            wst_sb.append(t)
        wfc_sb = consts.tile([128, J], F32, tag="wfc")
        nc.sync.dma_start(out=wfc_sb, in_=wfc_d)
        eye_sb = consts.tile([128, S * 128], MMD, tag="eye")
        nc.sync.dma_start(out=eye_sb, in_=eye_d)
        eye3 = eye_sb[:].rearrange("p (s q) -> p s q", s=S)

        import os

        nbt_run = int(os.environ.get("K_NBT", NBT))
        for bt in range(nbt_run):
            fea_sb = feap.tile([128, VGP], MMD, tag="feaT")
            nc.sync.dma_start(out=fea_sb, in_=fea_d[bt].rearrange("p a b -> p (a b)"))
            lab_sb = sm.tile([128, S], F32, tag="lab")
            nc.sync.dma_start(out=lab_sb, in_=lab_d[bt])
            emb_sb = embp.tile([128, S * D], MMD, tag="emb")
            nc.sync.dma_start(out=emb_sb, in_=emb_d[bt * 128 : (bt + 1) * 128])

            # fea_emb = fea @ W_emb.T  (no bias: b_emb fused into wst ones-row)
            ps_fe = ps_fe_p.tile([128, J], F32, tag="psfe")
            for k in range(NK):
                nc.tensor.matmul(
                    ps_fe,
                    fea_sb[:, k * 128 : (k + 1) * 128],
                    wet_sb[k][:],
                    start=(k == 0),
                    stop=(k == NK - 1),
                )
            fe_sb = fep.tile([128, J], MMD, tag="fe")
            nc.vector.tensor_scalar(fe_sb, ps_fe, 1.0, None, mul)
            stage = os.environ.get("K_STAGE", "full")
            if stage == "fe":
                dbg = outp.tile([128, D], F32, tag="ot")
                nc.vector.tensor_copy(dbg, fe_sb[:, 0:D])
                nc.sync.dma_start(out=out_d[bt], in_=dbg)
                continue

            # masks: nz = labels != 0 ; valid = prefix-AND(nz)
            nz = sm.tile([128, S], F32, tag="nz")
            nc.vector.tensor_scalar(nz, lab_sb, 0.0, None, mybir.AluOpType.not_equal)
            va = sm.tile([128, S], F32, tag="va")
            vb = sm.tile([128, S], F32, tag="vb")
            nc.vector.tensor_copy(va, nz)
            cur, nxt = va, vb
            for k in (1, 2, 4, 8, 16):
                nc.vector.tensor_copy(nxt[:, :k], cur[:, :k])
                nc.vector.tensor_tensor(
                    out=nxt[:, k:S], in0=cur[:, k:S], in1=cur[:, 0 : S - k], op=mul
                )
                cur, nxt = nxt, cur
            valid = cur

            # valid-diag: vd[p, s*128+q] = (p==q) * valid[p, s]
            vd = dgp.tile([128, S * 128], MMD, tag="vd")
            nc.vector.tensor_tensor(
                out=vd[:].rearrange("p (s q) -> p s q", s=S),
                in0=eye3,
                in1=valid[:].unsqueeze(2).broadcast_to([128, S, 128]),
                op=mul,
            )

            if stage == "mask":
                dbg = outp.tile([128, D], F32, tag="ot")
                nc.vector.tensor_copy(dbg, vd[:, 0:D])
                nc.sync.dma_start(out=out_d[bt], in_=dbg)
                continue
            scores = sm.tile([128, S], F32, tag="scores")
            for h in range(S // SH):
                cxs = []
                for i, (o, p) in enumerate(DP):
                    t = ctxp.tile([p, SH * 128], MMD, tag=f"cx{i}")
                    nc.sync.dma_start(
                        out=t,
                        in_=ctx_d[bt, o : o + p, h * SH : (h + 1) * SH, :].rearrange(
                            "p s b -> p (s b)"
                        ),
                    )
                    cxs.append(t)
                for si in range(SH):
                    s = h * SH + si
                    ps_h = ps_h_p.tile([128, J], F32, tag="psh")
                    for i in range(3):
                        nc.tensor.matmul(
                            ps_h,
                            cxs[i][:, si * 128 : (si + 1) * 128],
                            wst_sb[i][:],
                            start=(i == 0),
                            stop=False,
                        )
                    nc.tensor.matmul(
                        ps_h,
                        vd[:, s * 128 : (s + 1) * 128],
                        fe_sb[:],
                        start=False,
                        stop=True,
                    )
                    ht = hp.tile([128, J], F32, tag="H")
                    nc.scalar.activation(ht, ps_h, mybir.ActivationFunctionType.Tanh)
                    if stage == "tanh":
                        continue
                    hw = hp.tile([128, J], F32, tag="HW")
                    nc.vector.tensor_tensor(out=hw, in0=ht, in1=wfc_sb[:], op=mul)
                    nc.vector.tensor_reduce(
                        scores[:, s : s + 1], hw[:], axis=mybir.AxisListType.X,
                        op=add,
                    )

            if stage in ("scores", "tanh"):
                dbg = outp.tile([128, D], F32, tag="ot")
                nc.vector.memset(dbg, 0.0)
                if stage == "scores":
                    nc.vector.tensor_copy(dbg[:, 0:S], scores[:])
                nc.sync.dma_start(out=out_d[bt], in_=dbg)
                continue
            # masked softmax over s, renormalized
            negm = sm.tile([128, 1], F32, tag="negm")
            nc.vector.tensor_reduce(
                negm, scores[:], axis=mybir.AxisListType.X,
                op=mybir.AluOpType.max, negate=True,
            )
            e = sm.tile([128, S], F32, tag="e")
            nc.scalar.activation(
                e, scores[:], mybir.ActivationFunctionType.Exp, bias=negm[:, 0:1]
            )
            emk = sm.tile([128, S], F32, tag="emk")
            den = sm.tile([128, 1], F32, tag="den")
            nc.vector.tensor_tensor(out=emk, in0=e[:], in1=nz[:], op=mul)
            nc.vector.tensor_reduce(den, emk[:], axis=mybir.AxisListType.X, op=add)
            rden = sm.tile([128, 1], F32, tag="rden")
            nc.vector.reciprocal(rden, den)
            attn = sm.tile([128, S], F32, tag="attn")
            nc.vector.tensor_scalar(attn, emk, rden[:, 0:1], None, mul)

            if stage == "softmax":
                dbg = outp.tile([128, D], F32, tag="ot")
                nc.vector.memset(dbg, 0.0)
                nc.vector.tensor_copy(dbg[:, 0:S], attn[:])
                nc.sync.dma_start(out=out_d[bt], in_=dbg)
                continue
            # attn-diag + weighted sum of embedded, accumulated in PSUM
            ad = dgp.tile([128, S * 128], MMD, tag="ad")
            nc.vector.tensor_tensor(
                out=ad[:].rearrange("p (s q) -> p s q", s=S),
                in0=eye3,
                in1=attn[:].unsqueeze(2).broadcast_to([128, S, 128]),
                op=mul,
            )
            ps_wc = ps_wc_p.tile([128, D], F32, tag="pswc")
            for s in range(S):
                nc.tensor.matmul(
                    ps_wc,
                    ad[:, s * 128 : (s + 1) * 128],
                    emb_sb[:, s * D : (s + 1) * D],
                    start=(s == 0),
                    stop=(s == S - 1),
                )
            ot = outp.tile([128, D], F32, tag="ot")
            nc.scalar.copy(ot, ps_wc)
            nc.sync.dma_start(out=out_d[bt], in_=ot)

    nc.compile()
    return nc


def _get_nc(mm_dt_name="float32r"):
    if mm_dt_name not in _NC_CACHE:
        _NC_CACHE[mm_dt_name] = _build(getattr(mybir.dt, mm_dt_name))
    return _NC_CACHE[mm_dt_name]


def prep_inputs(context, embedded, input_labels, guide_input, sent_to_image_ann,
                W_sent, b_sent, W_emb, b_emb, w_fc, b_fc):
    """Host-side shard + layout prep. Pure data movement plus weight layout."""
    context = np.asarray(context, np.float32)
    embedded = np.asarray(embedded, np.float32)
    labels = np.asarray(input_labels)
    guide = np.asarray(guide_input, np.float32)
    ann2 = np.asarray(sent_to_image_ann)[::2]
    fea = guide[ann2]  # (B, VG) row gather

    wst = np.empty((DX, J), np.float32)
    wst[:D] = np.asarray(W_sent, np.float32).T
    wst[D] = np.asarray(b_sent, np.float32) + np.asarray(b_emb, np.float32)
    wet = np.zeros((VGP, J), np.float32)
    wet[:VG] = np.asarray(W_emb, np.float32).T
    wet = wet.reshape(NK, 128, J)
    wfc = np.tile(np.asarray(w_fc, np.float32)[None, :], (128, 1))
    eye = np.ascontiguousarray(
        np.tile(np.eye(128, dtype=np.float32), (1, S)).reshape(128, S * 128)
    )

    in_maps = []
    for c in range(NCORES):
        c0 = c * BC
        ctx_c = context[c0 : c0 + BC].reshape(NBT, 128, S, D).transpose(0, 3, 2, 1)
        ctx_l = np.empty((NBT, DX, S, 128), np.float32)
        ctx_l[:, :D] = ctx_c
        ctx_l[:, D] = 1.0
        fea_c = np.zeros((BC, VGP), np.float32)
        fea_c[:, :VG] = fea[c0 : c0 + BC]
        fea_l = fea_c.reshape(NBT, 128, NK, 128).transpose(0, 3, 2, 1)
        in_maps.append({
            "ctx": np.ascontiguousarray(ctx_l),
            "emb": np.ascontiguousarray(embedded[c0 : c0 + BC].reshape(BC, S * D)),
            "feaT": np.ascontiguousarray(fea_l),
            "lab": labels[c0 : c0 + BC].reshape(NBT, 128, S).astype(np.float32),
            "wst": wst, "wet": wet, "wfc": wfc, "eye": eye,
        })
    return in_maps


def kernel(**inputs):
    in_maps = prep_inputs(**inputs)
    nc = _get_nc()
    res = run_bass_kernel_spmd(nc, in_maps, list(range(NCORES)))
    return np.concatenate(
        [res.results[i]["wc"].reshape(BC, D) for i in range(NCORES)], axis=0
    )



# revision 9
# speedup vs baseline: 1.0222x; 1.0222x over previous
"""ContextAttention Trainium2 kernel (8-core data parallel, fp8 DoubleRow).

Computation (per batch row b, S=20, D=300, J=512):
  valid = cumprod(labels != 0)                      prefix-valid mask
  fea   = guide[ann[2b]]                            (host gather, pure data movement)
  pre[s,:] = ctx[b,s,:] @ W_sent.T + b_sent + b_emb + valid[b,s]*(fea @ W_emb.T)
  H = tanh(pre);  scores = H @ w_fc                 (b_fc dropped: softmax shift-invariant)
  attn = renorm(softmax(scores) * (labels != 0))
  out[b,:] = sum_s attn[s] * embedded[b,s,:]

Device layout: batch-major 128-row tiles.  Matmuls run in fp8e4 DoubleRow
(two 128-deep contraction subtiles per instruction at 0.5 cycles/row);
host pre-scales weights x64 to dodge fp8 subnormals, tanh un-scales.
Per s the pre-activation needs contraction depth 301(ctx+bias) + 128
(valid-diag guidance): packed into exactly TWO DoubleRow matmuls by
carrying the ctx tail (d>=256 + bias row) in the guidance matmul's
second subtile (vd4 sub1 = ctx tail, fe8 sub1 = W_sent tail).  The
attn-weighted sum contracts two s per instruction the same way.  tanh is
batched over 4 s in 4-bank PSUM tiles; scores = H @ w_fc runs as f16
multiplies (split DVE/GpSimd) + batched DVE reduces.
"""

import sys
from contextlib import ExitStack

import numpy as np

if "/opt/trn_rl_repo" not in sys.path:
    sys.path.append("/opt/trn_rl_repo")

import concourse.bass as bass
import concourse.tile as tile
from concourse import bacc, mybir
from concourse.bass_utils import run_bass_kernel_spmd

B, S, D, J, VG = 8192, 20, 300, 512, 2078
NCORES = 8
BC = B // NCORES          # 1024 batch rows per core
NBT = BC // 128           # 8 batch tiles per core
WSC = 64.0                # host weight pre-scale (fp8 subnormal avoidance)
NG = S // 4               # 4-s groups per tile
DVE_GROUPS = (0, 2, 4)    # score-multiply groups on DVE; rest on gpsimd
NT = 45                   # ctx tail rows: d=256..299 plus the bias-ones row
F32 = mybir.dt.float32
F16 = mybir.dt.float16
F8 = mybir.dt.float8e4
DR = mybir.MatmulPerfMode.DoubleRow

_NC_CACHE = {}


def _build():
    nc = bacc.Bacc("TRN2", target_bir_lowering=False, debug=False)

    ctxm_d = nc.dram_tensor("ctxm", [NBT, 128, 2 * S * 128], F8, kind="ExternalInput").ap()
    ctxt_d = nc.dram_tensor("ctxt", [NBT, NT, S * 128], F8, kind="ExternalInput").ap()
    feam_d = nc.dram_tensor("feam", [NBT, 128, 2 * 8 * 128], F8, kind="ExternalInput").ap()
    feat_d = nc.dram_tensor("feat", [NBT, 16, 2 * 128], F8, kind="ExternalInput").ap()
    emb_d = nc.dram_tensor("emb", [NBT, 128, S * D], F16, kind="ExternalInput").ap()
    lab_d = nc.dram_tensor("lab", [NBT, 128, S], F32, kind="ExternalInput").ap()
    wstm_d = nc.dram_tensor("wstm", [128, 2 * J], F8, kind="ExternalInput").ap()
    wstt_d = nc.dram_tensor("wstt", [128, J], F8, kind="ExternalInput").ap()
    wetm_d = nc.dram_tensor("wetm", [8, 128, 2 * J], F8, kind="ExternalInput").ap()
    wett_d = nc.dram_tensor("wett", [16, 2 * J], F8, kind="ExternalInput").ap()
    wfc4_d = nc.dram_tensor("wfc4", [128, 4 * J], F16, kind="ExternalInput").ap()
    eyeg_d = nc.dram_tensor("eyeg", [128, 128], F8, kind="ExternalInput").ap()
    eye_d = nc.dram_tensor("eye", [128, 128], F8, kind="ExternalInput").ap()
    out_d = nc.dram_tensor("wc", [NBT, 128, D], F32, kind="ExternalOutput").ap()

    mul = mybir.AluOpType.mult
    add = mybir.AluOpType.add

    with tile.TileContext(nc) as tc, ExitStack() as ctx:
        consts = ctx.enter_context(tc.tile_pool(name="consts", bufs=1))
        ctxp = ctx.enter_context(tc.tile_pool(name="ctxp", bufs=3))
        feap = ctx.enter_context(tc.tile_pool(name="feap", bufs=2))
        embp = ctx.enter_context(tc.tile_pool(name="embp", bufs=2))
        hp = ctx.enter_context(tc.tile_pool(name="hp", bufs=3))
        hwp = ctx.enter_context(tc.tile_pool(name="hwp", bufs=3))
        adp = ctx.enter_context(tc.tile_pool(name="adp", bufs=2))
        sm = ctx.enter_context(tc.tile_pool(name="sm", bufs=2))
        outp = ctx.enter_context(tc.tile_pool(name="outp", bufs=2))
        psp = ctx.enter_context(tc.tile_pool(name="psp", bufs=2, space="PSUM"))

        # ---- constants ----
        wstm_sb = consts.tile([128, 2, J], F8, tag="wstm")
        nc.sync.dma_start(out=wstm_sb, in_=wstm_d.rearrange("p (a j) -> p a j", a=2))
        wetm_sb = []
        for k in range(8):
            t = consts.tile([128, 2, J], F8, tag=f"wetm{k}")
            nc.sync.dma_start(out=t, in_=wetm_d[k].rearrange("p (a j) -> p a j", a=2))
            wetm_sb.append(t)
        wett_sb = consts.tile([16, 2, J], F8, tag="wett")
        nc.sync.dma_start(out=wett_sb, in_=wett_d.rearrange("p (a j) -> p a j", a=2))
        wfc4_sb = consts.tile([128, 4 * J], F16, tag="wfc4")
        nc.sync.dma_start(out=wfc4_sb, in_=wfc4_d)
        eyeg_sb = consts.tile([128, 128], F8, tag="eyeg")
        nc.sync.dma_start(out=eyeg_sb, in_=eyeg_d)
        eye_sb = consts.tile([128, 128], F8, tag="eye")
        nc.sync.dma_start(out=eye_sb, in_=eye_d)

        # fe8: sub0 = fea_emb (per tile, x8), sub1 = W_sent tail rows (const)
        # -> the guidance matmul's two subtiles add valid*fe*64 AND the ctx
        # tail (d>=256 incl. bias) in one instruction.
        fe8 = [consts.tile([128, 2, J], F8, name=f"fe8{i}", tag=f"fe8{i}")
               for i in range(2)]
        for i in range(2):
            nc.sync.dma_start(out=fe8[i][:, 1, :], in_=wstt_d)
        # vd4: sub0 = 8*valid-diag (DVE per tile), sub1 = ctx tail (DMA per
        # tile into rows 0..NT-1; rows NT..127 stay zero from the memset)
        vd4 = [consts.tile([128, S, 2, 128], F8, name=f"vd4{i}", tag=f"vd4{i}")
               for i in range(2)]
        for i in range(2):
            nc.vector.memset(vd4[i][:, :, 1, :], 0.0)

        for bt in range(NBT):
            vd = vd4[bt % 2]
            fe = fe8[bt % 2]

            ctxm_sb = ctxp.tile([128, 2, S, 128], F8, tag="ctxm")
            nc.sync.dma_start(
                out=ctxm_sb, in_=ctxm_d[bt].rearrange("p (a s b) -> p a s b", a=2, s=S)
            )
            nc.sync.dma_start(
                out=vd[0:NT, :, 1, :],
                in_=ctxt_d[bt].rearrange("p (s b) -> p s b", s=S),
            )
            feam_sb = feap.tile([128, 2, 8, 128], F8, tag="feam")
            nc.sync.dma_start(
                out=feam_sb, in_=feam_d[bt].rearrange("p (a k b) -> p a k b", a=2, k=8)
            )
            feat_sb = feap.tile([16, 2, 128], F8, tag="feat")
            nc.sync.dma_start(
                out=feat_sb, in_=feat_d[bt].rearrange("p (a b) -> p a b", a=2)
            )
            emb_sb = embp.tile([128, S * D], F16, tag="emb")
            nc.sync.dma_start(out=emb_sb, in_=emb_d[bt])
            lab_sb = sm.tile([128, S], F32, tag="lab")
            nc.sync.dma_start(out=lab_sb, in_=lab_d[bt])

            # fe = fea @ W_emb.T; psum carries x64 (host-scaled wet), fe8
            # keeps x8 (fp8 range), the other x8 rides on the valid-diag
            ps_fe = psp.tile([128, 4 * J], F32, tag="ps")
            for k in range(8):
                nc.tensor.matmul(
                    ps_fe[:, 0:J], feam_sb[:, :, k, :], wetm_sb[k][:],
                    start=(k == 0), stop=False, perf_mode=DR,
                )
            nc.tensor.matmul(
                ps_fe[:, 0:J], feat_sb[:], wett_sb[:], start=False, stop=True,
                perf_mode=DR,
            )
            nc.scalar.activation(
                fe[:, 0, :], ps_fe[:, 0:J], mybir.ActivationFunctionType.Copy,
                scale=0.125,
            )

            # masks: nz = labels != 0 ; valid = cumprod(nz) via scan
            nz = sm.tile([128, S], F32, tag="nz")
            nc.vector.tensor_scalar(nz, lab_sb, 0.0, None, mybir.AluOpType.not_equal)
            valid = sm.tile([128, S], F32, tag="valid")
            nc.vector.tensor_tensor_scan(
                valid, nz, nz, initial=1.0, op0=mul, op1=mybir.AluOpType.bypass
            )

            # valid-diag (x8 eye; fp8 sub0 of the DoubleRow pair)
            nc.vector.tensor_tensor(
                out=vd[:, :, 0, :],
                in0=eyeg_sb[:].unsqueeze(1).broadcast_to([128, S, 128]),
                in1=valid[:].unsqueeze(2).broadcast_to([128, S, 128]),
                op=mul,
            )

            scores = sm.tile([128, S], F32, tag="scores")
            for g in range(NG):
                ps4 = psp.tile([128, 4 * J], F32, tag="ps")
                for si in range(4):
                    s = 4 * g + si
                    pss = ps4[:, si * J : (si + 1) * J]
                    nc.tensor.matmul(
                        pss, ctxm_sb[:, :, s, :], wstm_sb[:],
                        start=True, stop=False, perf_mode=DR,
                    )
                    nc.tensor.matmul(
                        pss, vd[:, s], fe[:], start=False, stop=True, perf_mode=DR,
                    )
                ht4 = hp.tile([128, 4 * J], F16, tag="ht4")
                nc.scalar.activation(
                    ht4, ps4, mybir.ActivationFunctionType.Tanh, scale=1.0 / WSC
                )
                eng = nc.vector if g in DVE_GROUPS else nc.gpsimd
                hw4 = hwp.tile([128, 4, J], F16, tag="hw4")
                eng.tensor_tensor(
                    out=hw4, in0=ht4[:].rearrange("p (a j) -> p a j", a=4),
                    in1=wfc4_sb[:].rearrange("p (a j) -> p a j", a=4), op=mul,
                )
                nc.vector.tensor_reduce(
                    scores[:, 4 * g : 4 * g + 4], hw4[:],
                    axis=mybir.AxisListType.X, op=add,
                )

            # masked softmax over s, renormalized
            negm = sm.tile([128, 1], F32, tag="negm")
            nc.vector.tensor_reduce(
                negm, scores[:], axis=mybir.AxisListType.X,
                op=mybir.AluOpType.max, negate=True,
            )
            e = sm.tile([128, S], F32, tag="e")
            nc.scalar.activation(
                e, scores[:], mybir.ActivationFunctionType.Exp, bias=negm[:, 0:1]
            )
            emk = sm.tile([128, S], F32, tag="emk")
            den = sm.tile([128, 1], F32, tag="den")
            nc.vector.tensor_tensor(out=emk, in0=e[:], in1=nz[:], op=mul)
            nc.vector.tensor_reduce(den, emk[:], axis=mybir.AxisListType.X, op=add)
            rden = sm.tile([128, 1], F32, tag="rden")
            nc.vector.reciprocal(rden, den)
            attn = sm.tile([128, S], F32, tag="attn")
            nc.vector.tensor_scalar(attn, emk, rden[:, 0:1], None, mul)

            # attn-diag (f16, gpsimd) + weighted sum of embedded
            ad = adp.tile([128, S, 128], F16, tag="ad")
            nc.gpsimd.tensor_tensor(
                out=ad,
                in0=eye_sb[:].unsqueeze(1).broadcast_to([128, S, 128]),
                in1=attn[:].unsqueeze(2).broadcast_to([128, S, 128]),
                op=mul,
            )
            ps_wc = psp.tile([128, 4 * J], F32, tag="ps")
            for s in range(S):
                nc.tensor.matmul(
                    ps_wc[:, 0:D], ad[:, s], emb_sb[:, s * D : (s + 1) * D],
                    start=(s == 0), stop=(s == S - 1),
                )
            ot = outp.tile([128, D], F32, tag="ot")
            nc.scalar.copy(ot, ps_wc[:, 0:D])
            nc.sync.dma_start(out=out_d[bt], in_=ot)

    nc.compile()
    return nc


def _get_nc():
    if "v3" not in _NC_CACHE:
        _NC_CACHE["v3"] = _build()
    return _NC_CACHE["v3"]


def prep_inputs(context, embedded, input_labels, guide_input, sent_to_image_ann,
                W_sent, b_sent, W_emb, b_emb, w_fc, b_fc):
    """Host-side shard + layout prep. Pure data movement plus weight layout."""
    f8 = mybir.dt.np(F8)
    f16 = np.float16
    context = np.asarray(context, np.float32)
    embedded = np.asarray(embedded, np.float32)
    labels = np.asarray(input_labels)
    guide = np.asarray(guide_input, np.float32)
    ann2 = np.asarray(sent_to_image_ann)[::2]
    fea = guide[ann2]  # (B, VG) row gather

    W_sent = np.asarray(W_sent, np.float32)
    W_emb = np.asarray(W_emb, np.float32)
    bias = np.asarray(b_sent, np.float32) + np.asarray(b_emb, np.float32)

    # W_sent.T x64: main d<256 as DoubleRow pairs [128,2,J] (d = sub*128+p);
    # tail rows d=256..299 + bias row into [128, J] (rows NT..127 zero) --
    # they ride as sub1 of the guidance matmul (fe8 sub1)
    wstm = np.ascontiguousarray(
        (W_sent.T[:256] * WSC).reshape(2, 128, J).transpose(1, 0, 2)
    )
    wstt = np.zeros((128, J), np.float32)
    wstt[: NT - 1] = W_sent.T[256:D] * WSC
    wstt[NT - 1] = bias * WSC

    # W_emb.T padded to 2080 rows, x64: main 8 chunks of 256 (vg =
    # kk*256 + sub*128 + p), tail [16,2,J] (vg = 2048 + sub*16 + p)
    wet = np.zeros((2080, J), np.float32)
    wet[:VG] = W_emb.T * WSC
    wetm = np.ascontiguousarray(
        wet[:2048].reshape(8, 2, 128, J).transpose(0, 2, 1, 3)
    )
    wett = np.ascontiguousarray(wet[2048:].reshape(2, 16, J).transpose(1, 0, 2))

    wfc4 = np.tile(np.asarray(w_fc, np.float32)[None, :], (128, 4))
    eye = np.eye(128, dtype=np.float32)

    wstm8 = wstm.astype(f8).reshape(128, 2 * J)
    wstt8 = wstt.astype(f8)
    wetm8 = wetm.astype(f8).reshape(8, 128, 2 * J)
    wett8 = wett.astype(f8).reshape(16, 2 * J)
    wfc416 = wfc4.astype(f16)
    eye8 = eye.astype(f8)
    eyeg8 = (8.0 * eye).astype(f8)

    in_maps = []
    for c in range(NCORES):
        c0 = c * BC
        # ctx contraction-major: main pairs [128,2,S,128]; tail rows
        # 256..299 + ones row as [NT, S, 128]
        ctx_c = context[c0 : c0 + BC].reshape(NBT, 128, S, D).transpose(0, 3, 2, 1)
        ctxm = np.ascontiguousarray(
            ctx_c[:, :256].reshape(NBT, 2, 128, S, 128).transpose(0, 2, 1, 3, 4)
        )
        ctxt = np.empty((NBT, NT, S, 128), np.float32)
        ctxt[:, : NT - 1] = ctx_c[:, 256:D]
        ctxt[:, NT - 1] = 1.0
        # fea: pad VG to 2080, transpose, pack pairs
        fea_c = np.zeros((BC, 2080), np.float32)
        fea_c[:, :VG] = fea[c0 : c0 + BC]
        fea_t = fea_c.reshape(NBT, 128, 2080).transpose(0, 2, 1)  # [NBT, vg, b]
        feam = fea_t[:, :2048].reshape(NBT, 8, 2, 128, 128).transpose(0, 3, 2, 1, 4)
        feat = fea_t[:, 2048:].reshape(NBT, 2, 16, 128).transpose(0, 2, 1, 3)
        in_maps.append({
            "ctxm": ctxm.astype(f8).reshape(NBT, 128, 2 * S * 128),
            "ctxt": ctxt.astype(f8).reshape(NBT, NT, S * 128),
            "feam": np.ascontiguousarray(feam).astype(f8).reshape(NBT, 128, 2 * 8 * 128),
            "feat": np.ascontiguousarray(feat).astype(f8).reshape(NBT, 16, 2 * 128),
            "emb": embedded[c0 : c0 + BC].astype(f16).reshape(NBT, 128, S * D),
            "lab": labels[c0 : c0 + BC].reshape(NBT, 128, S).astype(np.float32),
            "wstm": wstm8, "wstt": wstt8, "wetm": wetm8, "wett": wett8,
            "wfc4": wfc416, "eye": eye8, "eyeg": eyeg8,
        })
    return in_maps


def kernel(**inputs):
    in_maps = prep_inputs(**inputs)
    nc = _get_nc()
    res = run_bass_kernel_spmd(nc, in_maps, list(range(NCORES)))
    return np.concatenate(
        [res.results[i]["wc"].reshape(BC, D) for i in range(NCORES)], axis=0
    )
